# revision 19
# baseline (speedup 1.0000x reference)
"""Tensor-parallel causal multi-head attention (RoPE) for 8 Trainium2 cores.

Problem: nn_Attention (B=2, S=2048, E=2048, H=16, interleaved-pair RoPE,
causal softmax with 1/sqrt(E) scaling, output projection).

Sharding: tensor-parallel over heads — each of the 8 cores owns 2 heads
(the matching 256 columns of Wq/Wk/Wv and rows of Wo), x is replicated,
and the post-Wo all-reduce is done on the host (sum of 8 partials).

Per-core device pipeline (fp32 psum accumulation throughout):
  1. Q^T/K^T projections in transposed layout [D, t] as fp8e4 DoubleRow
     matmuls (k-tile pairs, 2x bf16 MAC rate; W pre-scaled by 64 and x by
     16 to clear the e4m3 subnormal range, the 1/(64*16)^2 undone inside
     the softmax exp).  V stays bf16 in natural layout [t, D] (the value
     path cannot afford fp8).  RoPE on the vector engine via
     host-precomputed cos/sin maps; head-dim de-interleaved (even feats
     then odd feats) through a host-side permutation of the Wq/Wk rows so
     rotation pairs are contiguous partitions.
  2. Attention per (batch, head) over q-tiles of 512 with 128-wide key
     chunks processed in pairs, software-pipelined two pairs deep: scores
     matmuls for pair p+2 are emitted before the exp/mask/PV consumption
     of pair p, so PE streams S|PV|D back-to-back while ACT exps in
     parallel (no max-subtraction: |scores/sqrt(E)| <~ 1.5 here).
     Diagonal chunks are causally trimmed to their valid N = 512-128*j
     query range and masked with the sliced j=0 triangle mask (DVE).
     The softmax denominator is one ones[128,128]-matmul per PAIR over a
     DVE bf16 pair-sum of the two exp chunks.
  3. Normalization: reciprocal_approx_fast(denom) then one DVE multiply
     fused into the out^T psum eviction.
  4. Output projection for all tokens AFTER both batches' attention (so
     its psum eviction does not contend with attention's ACT/DVE work),
     evicted on ACT which keeps up with the 4 matmuls per psum group,
     staged per token-chunk, one DMA each.  Host sums 8 partials in fp64.

Scheduling notes (measured): first-needed fp8 pieces are emitted ahead of
the bulk preamble; all matmul psum goes through one shared 2-bank tag
with a 3-deep rotation; ~3.5 us of dummy warm-up matmuls run during the
startup DMA window so real matmuls start at the full 2.4 GHz clock.
"""

import math
import os
from contextlib import ExitStack

import ml_dtypes
import numpy as np

import concourse.bass as bass
import concourse.mybir as mybir
import concourse.tile as tile
from concourse import bacc, bass_isa, bass_utils

# denominator strategy: "pe" = ones-matmul on TensorE into psum;
# "gpsimd" = accumulate exp chunks + partition_all_reduce on GpSimd
DENOM_MODE = os.environ.get("KERNEL_DENOM", "pe")
# partial-output dtype: bf16 halves the output DMA; host sums in fp64
OUT_BF16 = os.environ.get("KERNEL_OUT", "bf16") == "bf16"

# ---------------------------------------------------------------- constants
B, S, E = 2, 2048, 2048
H = 16
N_CORES = 8
HPC = H // N_CORES          # heads per core = 2
D = E // H                  # head dim = 128
T = B * S                   # tokens = 4096
HD = HPC * D                # per-core head dims = 256
ATTN_SCALE = 1.0 / math.sqrt(E)
ROPE_BASE = 10000.0
# q/k carry an SW*SX fp8 pre-scale each; undo it inside the exp
EXP_SCALE = ATTN_SCALE / float((64.0 * 16.0) ** 2)

P = 128
EC = E // P                 # 16 contraction chunks
T_TILE = 512
NT = T // T_TILE            # 8 projection token tiles
QTS = 512                   # attention q-tile size
NQT = S // QTS              # 4 q-tiles per (b, h)
NKC = S // P                # 16 key chunks per batch

BF16 = mybir.dt.bfloat16
F32 = mybir.dt.float32
F8E4 = mybir.dt.float8e4
NPBF16 = ml_dtypes.bfloat16
NPF8E4 = ml_dtypes.float8_e4m3

# fp8 pre-scales: lift W entries (std ~1/sqrt(E)) and x out of the e4m3
# subnormal range; 1/(SW*SX)^2 is folded into the exp scale
SW = 64.0
SX = 16.0
ECP = EC // 2                # 8 DoubleRow k-tile pairs


# ---------------------------------------------------------------- device IR
def _emit(tc, ctx):
    nc = tc.nc
    xTt = nc.dram_tensor("xTt", [NT, P, EC, T_TILE], BF16, kind="ExternalInput").ap()
    xT8 = nc.dram_tensor("xT8", [NT, P, EC, T_TILE], F8E4, kind="ExternalInput").ap()
    wq8 = nc.dram_tensor("wq8", [P, EC, HD], F8E4, kind="ExternalInput").ap()
    wk8 = nc.dram_tensor("wk8", [P, EC, HD], F8E4, kind="ExternalInput").ap()
    wvT = nc.dram_tensor("wvT", [P, EC, HD], BF16, kind="ExternalInput").ap()
    woT = nc.dram_tensor("woT", [P, HPC, E], BF16, kind="ExternalInput").ap()
    rm1 = nc.dram_tensor("rm1", [P, T], BF16, kind="ExternalInput").ap()
    rm2 = nc.dram_tensor("rm2", [P, T], BF16, kind="ExternalInput").ap()
    msk = nc.dram_tensor("msk", [P, 4, QTS], BF16, kind="ExternalInput").ap()
    out = nc.dram_tensor("out", [T, E], BF16 if OUT_BF16 else F32,
                         kind="ExternalOutput").ap()

    wpool = ctx.enter_context(tc.tile_pool(name="wpool", bufs=1))
    xpool = ctx.enter_context(tc.tile_pool(name="xpool", bufs=2))
    qkv = ctx.enter_context(tc.tile_pool(name="qkv", bufs=1))
    work = ctx.enter_context(tc.tile_pool(name="work", bufs=3))
    psA = ctx.enter_context(tc.tile_pool(name="psA", bufs=2, space="PSUM"))
    psO = ctx.enter_context(tc.tile_pool(name="psO", bufs=2, space="PSUM"))
    psD = ctx.enter_context(tc.tile_pool(name="psD", bufs=2, space="PSUM"))

    # --- persistent SBUF state
    wq_s = wpool.tile([P, EC, HD], F8E4)
    wk_s = wpool.tile([P, EC, HD], F8E4)
    wv_s = wpool.tile([P, EC, HD], BF16)
    wo_s = wpool.tile([P, HPC, E], BF16)
    m1_s = wpool.tile([P, T], BF16)
    m2_s = wpool.tile([P, T], BF16)
    mk_s = wpool.tile([P, 4, QTS], BF16)
    ones_s = wpool.tile([P, P], BF16)
    # startup-latency ordering: the fp8 pieces for the first Q psum group
    # (all 16 ec chunks of xt8[0] + wq8) go first; the bf16 x tile (for V)
    # and everything else stream behind the warm-up window
    q4sl = [slice(q * (EC // 4), (q + 1) * (EC // 4)) for q in range(4)]
    nc.any.memset(ones_s[:], 1.0)
    xt0 = xpool.tile([P, EC, T_TILE], BF16, tag="xt")
    x80 = xpool.tile([P, EC, T_TILE], F8E4, tag="x8")
    nc.sync.dma_start(x80[:], xT8[0, :, :, :])
    nc.sync.dma_start(wq_s[:, 0:8, :], wq8[:, 0:8, :])
    nc.sync.dma_start(m1_s[:, 0:T_TILE], rm1[:, 0:T_TILE])
    nc.sync.dma_start(m2_s[:, 0:T_TILE], rm2[:, 0:T_TILE])
    nc.sync.dma_start(wq_s[:, 8:, :], wq8[:, 8:, :])
    # HAM warm-up: ~3.5 µs of dummy matmuls during the startup DMA window
    # so the first real matmuls run at the full 2.4 GHz clock
    warm = psA.tile([P, 512], F32, tag="big", bufs=3,
                    padded_shape=[P, 2 * QTS])
    for i in range(32):
        nc.tensor.matmul(warm[:, 0:P], lhsT=ones_s[:], rhs=ones_s[:],
                         start=(i == 0), stop=(i == 31))
    nc.sync.dma_start(wk_s[:, 0:8, :], wk8[:, 0:8, :])
    nc.sync.dma_start(wk_s[:, 8:, :], wk8[:, 8:, :])
    nc.scalar.dma_start(xt0[:, 0:8, :], xTt[0, :, 0:8, :])
    nc.scalar.dma_start(xt0[:, 8:, :], xTt[0, :, 8:, :])
    nc.scalar.dma_start(wv_s[:, 0:8, :], wvT[:, 0:8, :])
    nc.scalar.dma_start(wv_s[:, 8:, :], wvT[:, 8:, :])

    qT_s = qkv.tile([P, HPC, T], BF16)   # roped Q^T  [d, h, t]
    kT_s = qkv.tile([P, HPC, T], BF16)   # roped K^T
    v_s = qkv.tile([P, T // P, HD], BF16)  # V natural [t%128, t//128, hd]
    oT_s = qkv.tile([P, HPC, T], BF16)   # normalized out^T [d, h, t]

    # ---------------- phase A: projections + RoPE
    # Q/K are fp8e4 DoubleRow matmuls: each instruction contracts a PAIR of
    # 128-deep k-tiles (lhsT [128,2,128], rhs [128,2,256] -> out [128,256])
    # at 2x the bf16 MAC rate.  The ecp loop is OUTER so the four psum
    # slices (2 heads x 2 halves) accumulate chunk-by-chunk as x8 DMAs land.
    for tt in range(NT):
        ts0 = tt * T_TILE
        if tt == 0:
            xt, x8 = xt0, x80
        else:
            xt = xpool.tile([P, EC, T_TILE], BF16, tag="xt")
            x8 = xpool.tile([P, EC, T_TILE], F8E4, tag="x8")
            nc.sync.dma_start(x8[:, 0:8, :], xT8[tt, :, 0:8, :])
            nc.sync.dma_start(x8[:, 8:, :], xT8[tt, :, 8:, :])
            # just-in-time rope-map slice for this tile's tokens
            nc.sync.dma_start(m1_s[:, ts0:ts0 + T_TILE], rm1[:, ts0:ts0 + T_TILE])
            nc.sync.dma_start(m2_s[:, ts0:ts0 + T_TILE], rm2[:, ts0:ts0 + T_TILE])
            nc.scalar.dma_start(xt[:, 0:8, :], xTt[tt, :, 0:8, :])
            nc.scalar.dma_start(xt[:, 8:, :], xTt[tt, :, 8:, :])
            if tt == 4:
                nc.sync.dma_start(mk_s[:], msk[:])
            if tt == NT - 1:
                nc.scalar.dma_start(wo_s[:], woT[:])

        for w_s, dst in ((wq_s, qT_s), (wk_s, kT_s)):
            psb = psA.tile([P, 2 * T_TILE], F32, tag="big", bufs=3)
            for hs in range(HPC):
                for nhf in range(2):
                    for ecp in range(ECP):
                        nc.tensor.matmul(
                            psb[:, hs * T_TILE + nhf * 256:
                                hs * T_TILE + (nhf + 1) * 256],
                            lhsT=w_s[:, 2 * ecp:2 * ecp + 2,
                                     hs * P:(hs + 1) * P],
                            rhs=x8[:, 2 * ecp:2 * ecp + 2,
                                   nhf * 256:(nhf + 1) * 256],
                            start=(ecp == 0),
                            stop=(ecp == ECP - 1),
                            perf_mode=mybir.MatmulPerfMode.DoubleRow,
                        )
            for hs in range(HPC):
                ps = psb[:, hs * T_TILE:(hs + 1) * T_TILE]
                # RoPE: e = [x1; x2], swp = [x2; x1] (half-swap via DMA);
                # out = e*[cos;cos] + swp*[-sin;sin]
                e_t = work.tile([P, T_TILE], BF16, tag="rope_e")
                nc.scalar.copy(e_t[:], ps)
                swp = work.tile([P, T_TILE], BF16, tag="rope_s")
                nc.sync.dma_start(swp[0:64, :], e_t[64:128, :])
                nc.sync.dma_start(swp[64:128, :], e_t[0:64, :])
                a_t = work.tile([P, T_TILE], BF16, tag="rope_a")
                b_t = work.tile([P, T_TILE], BF16, tag="rope_b")
                nc.vector.tensor_mul(a_t[:], e_t[:], m1_s[:, ts0:ts0 + T_TILE])
                nc.vector.tensor_mul(b_t[:], swp[:], m2_s[:, ts0:ts0 + T_TILE])
                nc.vector.tensor_add(dst[:, hs, ts0:ts0 + T_TILE], a_t[:], b_t[:])

        for sp in range(T_TILE // P // 2):
            psb = psA.tile([P, 2 * HD], F32, tag="big", bufs=3,
                           padded_shape=[P, 2 * QTS])
            for k in range(2):
                sub = 2 * sp + k
                for ec in range(EC):
                    nc.tensor.matmul(
                        psb[:, k * HD:(k + 1) * HD],
                        lhsT=xt[:, ec, sub * P:(sub + 1) * P],
                        rhs=wv_s[:, ec, :],
                        start=(ec == 0),
                        stop=(ec == EC - 1),
                    )
            nc.scalar.copy(
                v_s[:, tt * (T_TILE // P) + 2 * sp:
                    tt * (T_TILE // P) + 2 * sp + 2, :], psb[:])

    # ---------------- phase B: attention per (batch, head)
    # Software-pipelined: the scores matmuls for pair p+1 are emitted ahead
    # of the exp/mask/PV consumption of pair p, so the PE streams
    # S(p+1) | PV(p) | D(p) back-to-back while ACT exps pair p in parallel.
    # The softmax denominator is computed with one ones-matmul per PAIR over
    # a DVE bf16 pair-sum of the two exp chunks (halves the PE denominator
    # work and instruction count vs per-chunk ones-matmuls).
    for b in range(B):
        for hs in range(HPC):
            qTb = qT_s[:, hs, b * S:(b + 1) * S]
            kTb = kT_s[:, hs, b * S:(b + 1) * S]
            for qt in range(NQT):
                q0 = qt * QTS
                nck = (q0 + QTS) // P  # causal: key chunks 0..nck-1
                npair = nck // 2
                ops = psO.tile([P, QTS], F32, tag="outT", bufs=1)
                dps = psD.tile([P, QTS], F32, tag="den", bufs=1)

                pending = []  # (sps, cc, off)

                def emit_scores(pp):
                    cc = (2 * pp, 2 * pp + 1)
                    # causal trim: diagonal chunk j (=c-(nck-4)) only has
                    # valid queries q >= q0 + 128*j  ->  width 512-128*j
                    off = [128 * max(0, c - (nck - 4)) for c in cc]
                    sps = psA.tile([P, 2 * QTS], F32, tag="big", bufs=3)
                    for half, c in enumerate(cc):
                        nc.tensor.matmul(
                            sps[:, half * QTS + off[half]:(half + 1) * QTS],
                            lhsT=kTb[:, c * P:(c + 1) * P],
                            rhs=qTb[:, q0 + off[half]:q0 + QTS],
                            start=True,
                            stop=True,
                        )
                    pending.append((sps, cc, off))

                def consume():
                    sps, cc, off = pending.pop(0)
                    ex = work.tile([P, 2 * QTS], BF16, tag="exps", bufs=6)
                    if off[0] == 0 and off[1] == 0:
                        nc.scalar.activation(
                            ex[:], sps[:], mybir.ActivationFunctionType.Exp,
                            scale=EXP_SCALE,
                        )
                    else:
                        for half in range(2):
                            sl = slice(half * QTS + off[half], (half + 1) * QTS)
                            nc.scalar.activation(
                                ex[:, sl], sps[:, sl],
                                mybir.ActivationFunctionType.Exp,
                                scale=EXP_SCALE,
                            )
                    for half, c in enumerate(cc):
                        w = QTS - off[half]
                        exh = ex[:, half * QTS + off[half]:(half + 1) * QTS]
                        if c >= nck - 4:
                            # intra-block triangle: reuse the j=0 mask, width w
                            nc.vector.tensor_mul(exh, exh, mk_s[:, 0, :w])
                    # pair-sum for the denominator (bf16, one DVE op for the
                    # common full-width case)
                    dsum = work.tile([P, QTS], BF16, tag="dsum", bufs=3)
                    ex0 = ex[:, off[0]:QTS]
                    ex1 = ex[:, QTS + off[1]:2 * QTS]
                    if off[0] == off[1]:
                        nc.vector.tensor_add(dsum[:, off[0]:], ex0, ex1)
                    else:
                        nc.vector.tensor_copy(
                            out=dsum[:, off[0]:off[1]],
                            in_=ex[:, off[0]:off[1]])
                        nc.vector.tensor_add(
                            dsum[:, off[1]:], ex[:, off[1]:QTS], ex1)
                    for half, c in enumerate(cc):
                        exh = ex[:, half * QTS + off[half]:(half + 1) * QTS]
                        nc.tensor.matmul(
                            ops[:, off[half]:QTS],
                            lhsT=v_s[:, b * NKC + c, hs * P:(hs + 1) * P],
                            rhs=exh,
                            start=(c == 0),
                            stop=(c == nck - 1),
                        )
                    nc.tensor.matmul(
                        dps[:, off[0]:QTS],
                        lhsT=ones_s[:],
                        rhs=dsum[:, off[0]:],
                        start=(cc[0] == 0),
                        stop=(cc[1] == nck - 1),
                    )

                for pp in range(npair):
                    emit_scores(pp)
                    if pp >= 2:
                        consume()
                while pending:
                    consume()
                # normalize: oT = ops * (1/denom), denom replicated to all
                # 128 partitions by the ones-matmul
                oslice = oT_s[:, hs, b * S + q0: b * S + q0 + QTS]
                rb = work.tile([P, QTS], F32, tag="recipb")
                nc.vector.reciprocal_approx_fast(out=rb[:], in_=dps[:])
                nc.vector.tensor_mul(oslice, ops[:], rb[:])

    # ---------------- phase C: output projection (all tokens, after both
    # batches so its psum eviction doesn't contend with attention's
    # ACT/DVE work).  Eviction on ACT (853ns per [128,1024]) keeps up with
    # the 4 matmuls (~1052ns) per psum group, unlike DVE (~1357ns).
    for tch in range(T // P):
        t0 = tch * P
        stage = work.tile([P, E], BF16 if OUT_BF16 else F32, tag="wo_out")
        for ep in range(E // 1024):
            wps = psA.tile([P, 1024], F32, tag="big", bufs=3)
            for k in range(2):
                es = 2 * ep + k
                for hc in range(HPC):
                    nc.tensor.matmul(
                        wps[:, k * 512:(k + 1) * 512],
                        lhsT=oT_s[:, hc, t0:t0 + P],
                        rhs=wo_s[:, hc, es * 512:(es + 1) * 512],
                        start=(hc == 0),
                        stop=(hc == HPC - 1),
                    )
            nc.scalar.copy(
                out=stage[:, ep * 1024:(ep + 1) * 1024], in_=wps[:])
            if tch == T // P - 1:
                # drain the final tile per-slice to shorten the tail
                nc.sync.dma_start(
                    out[t0:t0 + P, ep * 1024:(ep + 1) * 1024],
                    stage[:, ep * 1024:(ep + 1) * 1024])
        if tch != T // P - 1:
            nc.sync.dma_start(out[t0:t0 + P, :], stage[:])


def build_nc():
    nc = bacc.Bacc("TRN2", target_bir_lowering=False, debug=False, num_devices=1)
    with tile.TileContext(nc) as tc, ExitStack() as ctx:
        _emit(tc, ctx)
    nc.compile()
    return nc


# ---------------------------------------------------------------- host prep
def _rope_maps():
    half = D // 2
    inv = 1.0 / (ROPE_BASE ** (np.arange(half, dtype=np.float64) / half))
    ang = np.arange(S, dtype=np.float64)[None, :] * inv[:, None]  # [64, S]
    cos = np.cos(ang)
    sin = np.sin(ang)
    m1 = np.concatenate([cos, cos], axis=0)   # [128, S] multiplies e=[x1;x2]
    m2 = np.concatenate([-sin, sin], axis=0)  # multiplies swp=[x2;x1]
    m1 = np.tile(m1, (1, B)).astype(NPBF16)   # [128, T] (t = b*S + s)
    m2 = np.tile(m2, (1, B)).astype(NPBF16)
    return np.ascontiguousarray(m1), np.ascontiguousarray(m2)


def _masks():
    kk = np.arange(P)[:, None]
    qq = np.arange(QTS)[None, :]
    m = np.stack([(kk + 128 * j <= qq) for j in range(4)], axis=1)
    return np.ascontiguousarray(m.astype(NPBF16))  # [128, 4, 512]


def _prep_in_maps(x, Wq, Wk, Wv, Wo):
    x = np.asarray(x, np.float32)
    Wq = np.asarray(Wq, np.float32)
    Wk = np.asarray(Wk, np.float32)
    Wv = np.asarray(Wv, np.float32)
    Wo = np.asarray(Wo, np.float32)

    # x^T tiled: [NT, 128, EC, T_TILE];  xT[e, t] = x[t, e]
    xT = x.reshape(T, E).T                                     # [E, T] f32
    xtt4 = np.ascontiguousarray(
        xT.reshape(EC, P, NT, T_TILE).transpose(2, 1, 0, 3))   # [NT,P,EC,TT]
    xtt = xtt4.astype(NPBF16)
    xt8 = (xtt4 * SX).astype(NPF8E4)

    m1, m2 = _rope_maps()
    msk = _masks()

    # de-interleave perm for RoPE pair-contiguity
    perm = np.concatenate([np.arange(0, D, 2), np.arange(1, D, 2)])

    def wslice(W, rows, np_dt, scale=1.0):
        # -> [P, EC, ncols] : wT[p, ec, c] = W[rows[c], ec*128 + p]
        wt = (W[rows].T * scale).astype(np_dt)   # [E, ncols]
        return np.ascontiguousarray(
            wt.reshape(EC, P, len(rows)).transpose(1, 0, 2))

    in_maps = []
    for core in range(N_CORES):
        heads = range(core * HPC, (core + 1) * HPC)
        rows_qk = np.concatenate([h * D + perm for h in heads])
        rows_v = np.concatenate([np.arange(h * D, (h + 1) * D) for h in heads])
        # woT[p, hc, e] = Wo[e, rows_v[hc*128 + p]]
        wo_t = Wo[:, rows_v].T.astype(NPBF16)    # [HD, E]
        wo_t = np.ascontiguousarray(
            wo_t.reshape(HPC, P, E).transpose(1, 0, 2))
        in_maps.append({
            "xTt": xtt,
            "xT8": xt8,
            "wq8": wslice(Wq, rows_qk, NPF8E4, SW),
            "wk8": wslice(Wk, rows_qk, NPF8E4, SW),
            "wvT": wslice(Wv, rows_v, NPBF16),
            "woT": wo_t,
            "rm1": m1,
            "rm2": m2,
            "msk": msk,
        })
    return in_maps


_NC_CACHE = None


def _get_nc():
    global _NC_CACHE
    if _NC_CACHE is None:
        _NC_CACHE = build_nc()
    return _NC_CACHE


def kernel(x, Wq, Wk, Wv, Wo, _want_trace=False):
    in_maps = _prep_in_maps(x, Wq, Wk, Wv, Wo)
    nc = _get_nc()
    trace = _want_trace or bool(os.environ.get("KERNEL_TRACE"))
    res = bass_utils.run_bass_kernel_spmd(
        nc, in_maps, core_ids=list(range(N_CORES)), trace=trace,
    )
    acc = np.zeros((T, E), np.float64)
    for c in range(N_CORES):
        acc += res.results[c]["out"].astype(np.float64)
    outv = acc.astype(np.float32).reshape(B, S, E)
    if _want_trace:
        return outv, res
    return outv

